# revision 10
# baseline (speedup 1.0000x reference)
"""NeRF coarse+fine renderer on 8 Trainium2 NeuronCores (Bass/Tile).

Strategy: pure data parallel over rays (B=32768 -> 4096 rays/core).
All jax.random draws are input-independent (fixed key 42) and precomputed on
host CPU.  The fine-sampling uniforms `u` are pre-sorted per ray on host (with
the pairing jitter `r2` permuted identically): the reference sorts
concat(z_coarse, z_fine) anyway, so only the multiset of fine z-values
matters, and sorted-u makes searchsorted indices non-decreasing.

Milestone 1: two device passes with a host-side per-ray sort in between.
  pass1: coarse MLP+composite, inverse-CDF sampling -> rgb_c, depth_c, z_c, z_f
  host : z_all = sort(concat(z_c, z_f))
  pass2: fine MLP+composite over z_all -> rgb_f, depth_f, w_fine

The tiny MLP (3->64->4) is evaluated without matmuls: layer 1 is rank-2 per
ray (pts = o + z*d), so relu(A_h + z*D_h) for a whole (128-ray x K-sample)
tile is ONE ScalarE activation with per-partition scale/bias per hidden unit;
layer 2 contractions are DVE scalar_tensor_tensor / tensor_tensor_reduce
accumulations.  Transmittance cumprod and the CDF cumsum are single
tensor_tensor_scan ops per tile.
"""

import os
import sys

import numpy as np

for _p in ("/opt/trn_rl_repo",):
    if _p not in sys.path and os.path.isdir(_p):
        sys.path.insert(0, _p)

import concourse.bass as bass
import concourse.bacc as bacc
import concourse.mybir as mybir
from concourse.tile import TileContext
from concourse.bass_utils import run_bass_kernel_spmd

F32 = mybir.dt.float32
AF = mybir.ActivationFunctionType
OP = mybir.AluOpType

B = 32768
KC = 64
KF = 64
HID = 64
NCORES = 8
BC = B // NCORES          # rays per core
P = 128                   # rays per tile (partition dim)
NTILES = BC // P          # ray-tiles per core
STEP = 1.0 / KC

LAST_RESULTS = []         # BassKernelResults of the most recent kernel() call


# --------------------------------------------------------------------------
# device program builders
# --------------------------------------------------------------------------

def _load_consts(nc, tc, pool):
    """Replicated parameter tiles (prepared host-side, shape (128, ...))."""
    c = {}
    for name, cols in (("w1r", 3 * HID), ("b1r", HID), ("w2sr", HID),
                       ("w2rgbr", 3 * HID), ("b2r", 4)):
        dram = nc.dram_tensor(name, [P, cols], F32, kind="ExternalInput")
        t = pool.tile([P, cols], F32, tag=name)
        nc.sync.dma_start(t[:], dram[:])
        c[name] = t
    zeros = pool.tile([P, 2 * KC], F32, tag="zeros")
    nc.vector.memset(zeros[:], 0.0)
    c["zeros"] = zeros
    return c


def _emit_AD(nc, wk, consts, ray_t):
    """A = o@W1 + b1, D = d@W1 as (128 rays, 64 h) via fused per-partition FMAs."""
    w1r, b1r = consts["w1r"], consts["b1r"]
    A = wk.tile([P, HID], F32, tag="A")
    D = wk.tile([P, HID], F32, tag="D")
    t1 = wk.tile([P, HID], F32, tag="ad_t1")
    t2 = wk.tile([P, HID], F32, tag="ad_t2")
    # A
    nc.vector.scalar_tensor_tensor(t1[:], w1r[:, 0:HID], ray_t[:, 0:1],
                                   b1r[:], op0=OP.mult, op1=OP.add)
    nc.vector.scalar_tensor_tensor(t2[:], w1r[:, HID:2 * HID], ray_t[:, 1:2],
                                   t1[:], op0=OP.mult, op1=OP.add)
    nc.vector.scalar_tensor_tensor(A[:], w1r[:, 2 * HID:3 * HID], ray_t[:, 2:3],
                                   t2[:], op0=OP.mult, op1=OP.add)
    # D
    nc.vector.tensor_scalar_mul(t1[:], w1r[:, 0:HID], ray_t[:, 3:4])
    nc.vector.scalar_tensor_tensor(t2[:], w1r[:, HID:2 * HID], ray_t[:, 4:5],
                                   t1[:], op0=OP.mult, op1=OP.add)
    nc.vector.scalar_tensor_tensor(D[:], w1r[:, 2 * HID:3 * HID], ray_t[:, 5:6],
                                   t2[:], op0=OP.mult, op1=OP.add)
    return A, D


def _emit_composite(nc, wk, rk, jk, consts, ray_t, z, K):
    """Full composite over z (128, K).  Returns (w, rgb, depth) tiles."""
    zeros = consts["zeros"]
    A, D = _emit_AD(nc, wk, consts, ray_t)

    relu = rk.tile([P, HID * K], F32, tag=f"relu{K}")
    for h in range(HID):
        nc.scalar.activation(relu[:, h * K:(h + 1) * K], z[:], AF.Relu,
                             bias=A[:, h:h + 1], scale=D[:, h:h + 1])

    # sigma[b,k] = sum_h w2s[h] * relu[b,h,k]   (serial ping-pong over h)
    pp0 = wk.tile([P, K], F32, tag="sg_pp0")
    pp1 = wk.tile([P, K], F32, tag="sg_pp1")
    w2sr = consts["w2sr"]
    src = zeros[:, 0:K]
    for h in range(HID):
        dst = (pp0, pp1)[h & 1]
        nc.vector.scalar_tensor_tensor(dst[:], relu[:, h * K:(h + 1) * K],
                                       w2sr[:, h:h + 1], src,
                                       op0=OP.mult, op1=OP.add)
        src = dst[:]
    sigma = src

    # relu(sigma + b2[3])
    sr = wk.tile([P, K], F32, tag="sr")
    nc.scalar.activation(sr[:], sigma, AF.Relu, bias=consts["b2r"][:, 3:4])

    # deltas
    deltas = wk.tile([P, K], F32, tag="deltas")
    nc.vector.tensor_sub(deltas[:, 0:K - 1], z[:, 1:K], z[:, 0:K - 1])
    nc.vector.tensor_sub(deltas[:, K - 1:K], ray_t[:, 7:8], z[:, K - 1:K])

    # alphas = 1 - exp(-deltas*sr);  as1 = (1 - alphas) + 1e-10
    t = wk.tile([P, K], F32, tag="cmp_t")
    nc.vector.tensor_mul(t[:], deltas[:], sr[:])
    e = wk.tile([P, K], F32, tag="cmp_e")
    nc.scalar.activation(e[:], t[:], AF.Exp, scale=-1.0)
    a = wk.tile([P, K], F32, tag="cmp_a")
    nc.vector.tensor_scalar(a[:], e[:], -1.0, 1.0, op0=OP.mult, op1=OP.add)
    as1 = wk.tile([P, K], F32, tag="cmp_as1")
    nc.vector.tensor_scalar(as1[:], a[:], -1.0, 1.0, op0=OP.mult, op1=OP.add)
    nc.vector.tensor_scalar_add(as1[:], as1[:], 1e-10)

    # T = inclusive cumprod(as1);  w_k = a_k * T_{k-1}  (T_{-1}=1)
    T = wk.tile([P, K], F32, tag="cmp_T")
    nc.vector.tensor_tensor_scan(T[:], as1[:], zeros[:, 0:K], 1.0,
                                 op0=OP.mult, op1=OP.add)
    w = wk.tile([P, K], F32, tag="cmp_w")
    nc.vector.tensor_copy(w[:, 0:1], a[:, 0:1])
    nc.vector.tensor_mul(w[:, 1:K], a[:, 1:K], T[:, 0:K - 1])

    # H[b,h] = sum_k w_k relu[b,h,k];  rgb_c = H @ W2[:, :3] + b2[:3]*sum(w)
    Hm = wk.tile([P, HID], F32, tag="Hm")
    for h in range(HID):
        junk = jk.tile([P, K], F32, tag="junkH")
        nc.vector.scalar_tensor_tensor(junk[:], relu[:, h * K:(h + 1) * K],
                                       1.0, w[:], op0=OP.mult, op1=OP.mult,
                                       accum_out=Hm[:, h:h + 1])
    rgb_acc = wk.tile([P, 3], F32, tag="rgb_acc")
    w2rgbr = consts["w2rgbr"]
    for cch in range(3):
        junk = jk.tile([P, HID], F32, tag="junkC")
        nc.vector.scalar_tensor_tensor(junk[:], Hm[:], 1.0,
                                       w2rgbr[:, cch * HID:(cch + 1) * HID],
                                       op0=OP.mult, op1=OP.mult,
                                       accum_out=rgb_acc[:, cch:cch + 1])
    wsum = wk.tile([P, 1], F32, tag="wsum")
    junk = jk.tile([P, K], F32, tag="junkW")
    nc.scalar.activation(junk[:], w[:], AF.Copy, accum_out=wsum[:])
    rgb = wk.tile([P, 3], F32, tag="rgb")
    nc.vector.scalar_tensor_tensor(rgb[:], consts["b2r"][:, 0:3], wsum[:],
                                   rgb_acc[:], op0=OP.mult, op1=OP.add)
    depth = wk.tile([P, 1], F32, tag="depth")
    junk = jk.tile([P, K], F32, tag="junkD")
    nc.vector.scalar_tensor_tensor(junk[:], w[:], 1.0, z[:],
                                   op0=OP.mult, op1=OP.mult,
                                   accum_out=depth[:])
    return w, rgb, depth


def build_pass1():
    nc = bacc.Bacc(None, target_bir_lowering=False)
    rays_d = nc.dram_tensor("rays", [BC, 8], F32, kind="ExternalInput")
    sc_d = nc.dram_tensor("s_c", [BC, KC], F32, kind="ExternalInput")
    us_d = nc.dram_tensor("u_s", [BC, KF], F32, kind="ExternalInput")
    r2_d = nc.dram_tensor("r2p64", [BC, KF], F32, kind="ExternalInput")
    rgbc_d = nc.dram_tensor("rgb_c", [BC, 3], F32, kind="ExternalOutput")
    depc_d = nc.dram_tensor("depth_c", [BC, 1], F32, kind="ExternalOutput")
    zc_d = nc.dram_tensor("z_c", [BC, KC], F32, kind="ExternalOutput")
    zf_d = nc.dram_tensor("z_f", [BC, KF], F32, kind="ExternalOutput")

    with TileContext(nc) as tc:
        with (tc.tile_pool(name="const", bufs=1) as cpool,
              tc.tile_pool(name="io", bufs=3) as io,
              tc.tile_pool(name="wk", bufs=3) as wk,
              tc.tile_pool(name="relu", bufs=2) as rk,
              tc.tile_pool(name="junk", bufs=4) as jk):
            consts = _load_consts(nc, tc, cpool)
            for t in range(NTILES):
                r = slice(t * P, (t + 1) * P)
                ray_t = io.tile([P, 8], F32, tag="ray_t")
                sc_t = io.tile([P, KC], F32, tag="sc_t")
                us_t = io.tile([P, KF], F32, tag="us_t")
                r2_t = io.tile([P, KF], F32, tag="r2_t")
                nc.sync.dma_start(ray_t[:], rays_d[r, :])
                nc.sync.dma_start(sc_t[:], sc_d[r, :])
                nc.sync.dma_start(us_t[:], us_d[r, :])
                nc.sync.dma_start(r2_t[:], r2_d[r, :])

                near_b = ray_t[:, 6:7].to_broadcast([P, KC])
                dn = wk.tile([P, 1], F32, tag="dn")
                nc.vector.tensor_sub(dn[:], ray_t[:, 7:8], ray_t[:, 6:7])
                z = wk.tile([P, KC], F32, tag="z")
                nc.vector.scalar_tensor_tensor(z[:], sc_t[:], dn[:], near_b,
                                               op0=OP.mult, op1=OP.add)

                w, rgb, depth = _emit_composite(nc, wk, rk, jk, consts,
                                                ray_t, z, KC)

                # ---- fine sampling ----
                wp = wk.tile([P, KC], F32, tag="wp")
                nc.vector.tensor_scalar_add(wp[:], w[:], 1e-5)
                S = wk.tile([P, 1], F32, tag="S")
                junk = jk.tile([P, KC], F32, tag="junkS")
                nc.scalar.activation(junk[:], wp[:], AF.Copy, accum_out=S[:])
                inv = wk.tile([P, 1], F32, tag="inv")
                nc.vector.reciprocal(inv[:], S[:])
                pdf = wk.tile([P, KC], F32, tag="pdf")
                nc.vector.tensor_scalar_mul(pdf[:], wp[:], inv[:])
                cdf = wk.tile([P, KC], F32, tag="cdf")
                nc.vector.tensor_tensor_scan(cdf[:], pdf[:],
                                             consts["zeros"][:, 0:KC], 0.0,
                                             op0=OP.add, op1=OP.add)

                # inds = sum_{j=1..64} 1[u_s >= cdf_j]
                ss0 = wk.tile([P, KF], F32, tag="ss0")
                ss1 = wk.tile([P, KF], F32, tag="ss1")
                src = consts["zeros"][:, 0:KF]
                for j in range(KC):
                    dst = (ss0, ss1)[j & 1]
                    nc.vector.scalar_tensor_tensor(dst[:], us_t[:],
                                                   cdf[:, j:j + 1], src,
                                                   op0=OP.is_ge, op1=OP.add)
                    src = dst[:]

                # z_f = near + dn * (inds/64 + r2p/64)
                sf = wk.tile([P, KF], F32, tag="sf")
                nc.vector.scalar_tensor_tensor(sf[:], src, STEP, r2_t[:],
                                               op0=OP.mult, op1=OP.add)
                zf = wk.tile([P, KF], F32, tag="zf")
                nc.vector.scalar_tensor_tensor(zf[:], sf[:], dn[:],
                                               ray_t[:, 6:7].to_broadcast([P, KF]),
                                               op0=OP.mult, op1=OP.add)

                nc.sync.dma_start(rgbc_d[r, :], rgb[:])
                nc.sync.dma_start(depc_d[r, :], depth[:])
                nc.sync.dma_start(zc_d[r, :], z[:])
                nc.sync.dma_start(zf_d[r, :], zf[:])
    nc.compile()
    return nc


def build_pass2():
    K = KC + KF
    nc = bacc.Bacc(None, target_bir_lowering=False)
    rays_d = nc.dram_tensor("rays", [BC, 8], F32, kind="ExternalInput")
    za_d = nc.dram_tensor("z_all", [BC, K], F32, kind="ExternalInput")
    rgbf_d = nc.dram_tensor("rgb_f", [BC, 3], F32, kind="ExternalOutput")
    depf_d = nc.dram_tensor("depth_f", [BC, 1], F32, kind="ExternalOutput")
    wf_d = nc.dram_tensor("w_fine", [BC, K], F32, kind="ExternalOutput")

    with TileContext(nc) as tc:
        with (tc.tile_pool(name="const", bufs=1) as cpool,
              tc.tile_pool(name="io", bufs=3) as io,
              tc.tile_pool(name="wk", bufs=3) as wk,
              tc.tile_pool(name="relu", bufs=2) as rk,
              tc.tile_pool(name="junk", bufs=4) as jk):
            consts = _load_consts(nc, tc, cpool)
            for t in range(NTILES):
                r = slice(t * P, (t + 1) * P)
                ray_t = io.tile([P, 8], F32, tag="ray_t")
                z = io.tile([P, K], F32, tag="z_all")
                nc.sync.dma_start(ray_t[:], rays_d[r, :])
                nc.sync.dma_start(z[:], za_d[r, :])

                w, rgb, depth = _emit_composite(nc, wk, rk, jk, consts,
                                                ray_t, z, K)

                nc.sync.dma_start(rgbf_d[r, :], rgb[:])
                nc.sync.dma_start(depf_d[r, :], depth[:])
                nc.sync.dma_start(wf_d[r, :], w[:])
    nc.compile()
    return nc


# --------------------------------------------------------------------------
# host side
# --------------------------------------------------------------------------

_STATE = {}


_RAND_SRC = r'''
import sys
import numpy as np
import jax
B, KC, KF = 32768, 64, 64
key = jax.random.key(42)
kc, kf = jax.random.split(key)
u_c = np.asarray(jax.random.uniform(kc, (B, KC), dtype=np.float32))
k1, k2 = jax.random.split(kf)
u = np.asarray(jax.random.uniform(k1, (B, KF), dtype=np.float32))
r2 = np.asarray(jax.random.uniform(k2, (B, KF), dtype=np.float32))
np.savez(sys.argv[1], u_c=u_c, u=u, r2=r2)
'''


def _cpu_subprocess_randoms():
    """Draw the reference's uniforms with CPU jax in a scrubbed env — the
    axon/neuron backend produces different threefry results than the CPU
    backend the reference runs on."""
    import subprocess
    import tempfile
    env = dict(os.environ)
    env.pop("TRN_TERMINAL_POOL_IPS", None)
    env["JAX_PLATFORMS"] = "cpu"
    env["PYTHONPATH"] = os.pathsep.join(
        [p for p in sys.path if p] +
        [env[k] for k in ("NIX_PYTHONPATH",) if k in env])
    with tempfile.TemporaryDirectory() as td:
        f = os.path.join(td, "r.npz")
        subprocess.run([sys.executable, "-c", _RAND_SRC, f], env=env,
                       check=True, timeout=900)
        d = np.load(f)
        return d["u_c"], d["u"], d["r2"]


def _host_randoms():
    try:
        u_c, u, r2 = _cpu_subprocess_randoms()
    except Exception:
        import jax
        try:
            cpu = jax.devices("cpu")[0]
            ctx = jax.default_device(cpu)
        except Exception:
            import contextlib
            ctx = contextlib.nullcontext()
        with ctx:
            key = jax.random.key(42)
            kc, kf = jax.random.split(key)
            u_c = np.asarray(jax.random.uniform(kc, (B, KC), dtype=np.float32))
            k1, k2 = jax.random.split(kf)
            u = np.asarray(jax.random.uniform(k1, (B, KF), dtype=np.float32))
            r2 = np.asarray(jax.random.uniform(k2, (B, KF), dtype=np.float32))
    perm = np.argsort(u, axis=-1, kind="stable")
    u_s = np.take_along_axis(u, perm, -1)
    r2p = np.take_along_axis(r2, perm, -1)
    step = np.float32(STEP)
    lin = np.arange(KC, dtype=np.float32)[None, :] * step
    s_c = (lin + u_c * step).astype(np.float32)
    r2p64 = (r2p * step).astype(np.float32)
    return s_c, u_s, r2p64


def _get_state():
    if not _STATE:
        _STATE["rand"] = _host_randoms()
        _STATE["nc1"] = build_pass1()
        _STATE["nc2"] = build_pass2()
    return _STATE


def _rep(x, cols):
    """Host-side replicate a 1-D/2-D param across 128 partitions."""
    flat = np.ascontiguousarray(x, dtype=np.float32).reshape(1, cols)
    return np.broadcast_to(flat, (P, cols)).copy()


def kernel(rays, W1, b1, W2, b2):
    st = _get_state()
    s_c, u_s, r2p64 = st["rand"]
    rays = np.ascontiguousarray(rays, dtype=np.float32)

    w1r = _rep(np.asarray(W1, np.float32), 3 * HID)          # rows packed
    b1r = _rep(b1, HID)
    w2 = np.asarray(W2, np.float32)
    w2sr = _rep(w2[:, 3], HID)
    w2rgbr = _rep(w2[:, 0:3].T, 3 * HID)                     # per-channel rows
    b2r = _rep(b2, 4)
    consts = {"w1r": w1r, "b1r": b1r, "w2sr": w2sr,
              "w2rgbr": w2rgbr, "b2r": b2r}

    core_ids = list(range(NCORES))
    sl = [slice(c * BC, (c + 1) * BC) for c in range(NCORES)]

    in1 = [{"rays": rays[s], "s_c": s_c[s], "u_s": u_s[s], "r2p64": r2p64[s],
            **consts} for s in sl]
    res1 = run_bass_kernel_spmd(st["nc1"], in1, core_ids)
    LAST_RESULTS.clear()
    LAST_RESULTS.append(res1)
    r1 = res1.results

    rgb_c = np.concatenate([r["rgb_c"] for r in r1], 0)
    depth_c = np.concatenate([r["depth_c"] for r in r1], 0)[:, 0]
    z_c = np.concatenate([r["z_c"] for r in r1], 0)
    z_f = np.concatenate([r["z_f"] for r in r1], 0)

    z_all = np.sort(np.concatenate([z_c, z_f], -1), -1).astype(np.float32)

    in2 = [{"rays": rays[s], "z_all": z_all[s], **consts} for s in sl]
    res2 = run_bass_kernel_spmd(st["nc2"], in2, core_ids)
    LAST_RESULTS.append(res2)
    r2_ = res2.results

    rgb_f = np.concatenate([r["rgb_f"] for r in r2_], 0)
    depth_f = np.concatenate([r["depth_f"] for r in r2_], 0)[:, 0]
    w_fine = np.concatenate([r["w_fine"] for r in r2_], 0)

    return rgb_c, depth_c, rgb_f, depth_f, w_fine
